# revision 45
# baseline (speedup 1.0000x reference)
"""MultiHeadDualAttention Trainium2 kernel, v6 (eblk-granular software pipeline).

Sharding: 8 heads -> 8 cores. Each core: full k1/k2 (host-transposed to
[256, 4096] bf16) and v1/v2 (fp8e4 -- the v path tolerates the extra
quantization, relerr 1.57e-2 vs the 2e-2 gate, and it halves that DMA) + its
head's wk slices (column-duplicated on host so the k-projection matmul emits
both 64-row-group copies in one shot) and fp8 wv slices. Outputs per
core: unnormalized o1T/o2T [65, 4096] bf16 (row 64 = softmax denominator);
host divides, applies the wo projection (row-shard of wo = per-head slice,
concat over heads), and adds the v-bias/wo-bias constants (v-bias commutes
through softmax).

Math per head: S[m, n] = k2F[m]·k1F[n] (o2 direction; o1 swaps k1/k2) with
kF the *biased* projections; rowsoftmax / colsoftmax of the shared score
matrix are exactly the reference's two directions.

Perf structure (measured on HW via microbenchmarks):
  - unit of work = eblk: 2 score m-tiles x one 512-wide n-chunk.
    scores: 2 concurrent no-DR fp8 matmuls on 64-row groups (kf stores the
    projection twice, rows 0-63 / 64-127; warm pair issues in ~216ns).
    exp: full-tile [128,2,512] on ACT (exact Exp) or DVE (Schraudolph
    rn(S*a+b) -> int8 bitcast fp8e4), assigned by a greedy load balancer.
    PV: one fp8 DoubleRow full-array matmul (contract 256) accumulating
    [80,512]; row 64 (ones column) = softmax denominator.
  - software pipeline: PV runs 10 eblks behind its scores in the ramp (so a
    v-aug-gated PV cannot block the in-order PE queue), tapering to 5 in
    steady state; score-psum pool of 3 [128,2,512] tiles makes the
    write-after-read distance 3 eblks (the 8 PSUM banks allow no more).
  - steady cadence ~605-615ns/eblk = the ACT/DVE exp floor (both engines
    ~94% busy); the PE absorbs the k1/v1 projections in its slack.
  - HAM (PE clock-gate): warm bursts + ramp-only filler matmuls keep the
    2.4GHz gate engaged; in steady state the real work is dense enough.
    Filler matmuls double as WAR gates that hold back later raw-load
    transfers so early transfers get full HBM bandwidth.
  - wo projection + normalization on host.
"""

import sys

sys.path.insert(0, "/opt/trn_rl_repo")

import numpy as np

N = 4096
C = 256
D = 64
SCALE = float(D) ** -0.5
NCORES = 8
NCH = 8          # n-chunks of 512
EPC = 16         # eblks per chunk (16 m-pairs)
LAG = 5          # PV lag in eblks
A_SCH = float(8.0 * np.log2(np.e) * SCALE)   # schraudolph multiplier
B_SCH = 55.8                                  # schraudolph magic bias

_cache: dict = {}


def _build_module():
    import concourse.bacc as bacc
    import concourse.mybir as mybir
    import concourse.tile as tile

    f32 = mybir.dt.float32
    bf16 = mybir.dt.bfloat16
    fp8 = mybir.dt.float8e4
    i8 = mybir.dt.int8
    Exp = mybir.ActivationFunctionType.Exp
    Ident = mybir.ActivationFunctionType.Identity
    DR = mybir.MatmulPerfMode.DoubleRow
    Alu = mybir.AluOpType

    nc = bacc.Bacc("TRN2", target_bir_lowering=False, debug=False)

    def din(name, shape, dt=bf16):
        return nc.dram_tensor(name, shape, dt, kind="ExternalInput").ap()

    def dout(name, shape, dt):
        return nc.dram_tensor(name, shape, dt, kind="ExternalOutput").ap()

    k1T = din("k1T", [C, N])
    v1T = din("v1T", [C, N], mybir.dt.float8e4)
    k2T = din("k2T", [C, N])
    v2T = din("v2T", [C, N], mybir.dt.float8e4)
    wk1 = din("wk1", [C, 2 * D])          # column-duplicated on host
    wk2 = din("wk2", [C, 2 * D])
    wv1 = din("wv1", [C, D], mybir.dt.float8e4)
    wv2 = din("wv2", [C, D], mybir.dt.float8e4)
    bk1 = din("bk1", [2 * D, 1], f32)     # row-duplicated on host
    bk2 = din("bk2", [2 * D, 1], f32)

    o1Td = dout("o1T", [D + 1, N], bf16)
    o2Td = dout("o2T", [D + 1, N], bf16)

    # elementwise-engine load balancer (ns estimates from microbench)
    ew = {"act": 0.0, "dve": 0.0}

    def pick_engine(act_cost, dve_cost):
        if ew["act"] + act_cost <= ew["dve"] + dve_cost:
            ew["act"] += act_cost
            return "act"
        ew["dve"] += dve_cost
        return "dve"

    with tile.TileContext(nc) as tc:
        with (
            tc.tile_pool(name="const", bufs=1) as constp,
            tc.tile_pool(name="eblk", bufs=14) as ep,
            tc.tile_pool(name="outp", bufs=3) as outp,
            tc.tile_pool(name="spsum", bufs=3, space="PSUM") as spsum,
            tc.tile_pool(name="opsum", bufs=2, space="PSUM") as opsum,
        ):
            # ---- weights (gpsimd hwdge queue; SP queue is for raw loads) ----
            w_sb = {}
            _pending_w = []
            for name, drt, cols in (("wk1", wk1, 2 * D), ("wk2", wk2, 2 * D),
                                    ("wv1", wv1, D), ("wv2", wv2, D)):
                wdt = fp8 if name.startswith("wv") else bf16
                t = constp.tile([128, 2, cols], wdt, tag=name, name=f"w_{name}")
                for ct in range(2):
                    _pending_w.append((t[:, ct, :], drt[ct * 128:(ct + 1) * 128, :]))
                w_sb[name] = t
            bk1_sb = constp.tile([2 * D, 1], f32, tag="bk1")
            bk2_sb = constp.tile([2 * D, 1], f32, tag="bk2")

            warm = constp.tile([128, 512], bf16, tag="warm")
            nc.vector.memset(warm[:], 0.0)
            warm8 = constp.tile([64, 128], fp8, tag="warm8")
            nc.vector.memset(warm8[:], 0.0)

            # ---- raw tensors: one tile PER 2-chunk UNIT (tile-granular
            # dependency tracking would otherwise serialize gated loads of
            # the same tensor); c = ct*128 + p ----
            rawt = {(tg, u): constp.tile([128, 2, 2, 512],
                                         fp8 if tg.startswith("v") else bf16,
                                         tag=f"{tg}raw{u}", name=f"rawt_{tg}{u}")
                    for tg in ("k1", "k2", "v2", "v1") for u in range(NCH // 2)}
            rawd = {"k1": k1T, "k2": k2T, "v2": v2T, "v1": v1T}

            def load_unit(tg, u, eng):
                eng.dma_start(
                    out=rawt[(tg, u)][:],
                    in_=rawd[tg][:, (2 * u) * 512:(2 * u + 2) * 512].rearrange(
                        "(c p) (j n) -> p c j n", c=2, j=2))

            # raw loads split across TWO hwdge queues (SP + gpsimd): one
            # queue only sustains ~half the per-core HBM bandwidth, and
            # transfers within a queue share bandwidth round-robin — so only
            # the first-consumed units go in the initial batch; k1u1-3/v1
            # are emitted later on the gpsimd queue BEHIND the out-DMAs,
            # which gates their transfer start into the attention stream.
            for dst, srcw in _pending_w:
                nc.gpsimd.dma_start(out=dst, in_=srcw)
            nc.gpsimd.dma_start(out=bk1_sb[:], in_=bk1[:])
            nc.gpsimd.dma_start(out=bk2_sb[:], in_=bk2[:])
            load_unit("k1", 0, nc.sync)
            load_unit("k2", 0, nc.gpsimd)
            load_unit("v2", 0, nc.gpsimd)
            load_unit("k2", 1, nc.sync)
            # all other raw units are emitted inside the attention stream,
            # WAR-gated by a fill matmul that reads their destination region
            # (see s_and_exp gate), so early transfers get the full bandwidth

            # ---- kf staging: [128, N] fp8, rows 0-63 / 64-127 identical ----
            kf = {"k1": constp.tile([128, N], fp8, tag="k1f", name="kf_k1"),
                  "k2": constp.tile([128, N], fp8, tag="k2f", name="kf_k2")}

            def k_proj_chunk(tg, w, b_sb, j):
                """Project chunk j of tg into kf[tg] (both row copies). Uses an
                opsum-pool tile so the score-psum pool is never borrowed."""
                ps = opsum.tile([128, 512], f32, tag="po", name=f"kp_{tg}{j}")
                for ct in range(2):
                    nc.tensor.matmul(
                        ps[:], w[:, ct, :], rawt[(tg, j // 2)][:, ct, j % 2, :],
                        start=(ct == 0), stop=(ct == 1))
                nc.scalar.activation(
                    kf[tg][:, j * 512:(j + 1) * 512], ps[:], Ident, bias=b_sb[:])
                ew["act"] += 700.0

            # ---- v projections -> fp8 v-aug [128, pair, kt, 80] ----
            # per-unit granularity (2 chunks -> pairs 4u..4u+3) so v-aug
            # tracks the raw-load pipeline
            def v_proj_unit(raws, w, vaug, u):
                vps = opsum.tile([128, 512], f32, tag="po",
                                 name=f"vp_{id(vaug)}_{u}")
                for jj in range(2):
                    raw = raws[2 * u + jj]
                    for k in range(4):
                        nt_loc = jj * 4 + k
                        out = vps[:, nt_loc * D:(nt_loc + 1) * D]
                        for ct in range(2):
                            nc.tensor.matmul(
                                out, raw[:, ct, k * 128:(k + 1) * 128], w[:, ct, :],
                                start=(ct == 0), stop=(ct == 1))
                nc.vector.tensor_copy(
                    vaug[:, 4 * u:4 * (u + 1), :, 0:D], vps[:])
                ew["dve"] += 730.0

            def v_aug_alloc(tag):
                vaug = constp.tile([128, 16, 2, 80], fp8, tag=tag, name=f"vaug_{tag}")
                nc.vector.memset(vaug[:, :, :, D:80], 0.0)
                nc.vector.memset(vaug[:, :, :, D:D + 1], 1.0)
                return vaug

            # ---- HAM warm burst: dependency-free, issues during the raw
            # loads so the clock gate is open when eblk 0 starts ----
            wps = spsum.tile([128, 2, 512], f32, tag="sAB", name="warm_att")
            for _ in range(15):
                nc.tensor.matmul(wps[:, 0, :], warm[:, 0:128], warm[:],
                                 start=True, stop=True)

            # ---- pre-phase: eblk 0-7 inputs (k2u1 lands ~13us, before the
            # PE clears the warm bursts, so c2/c3 project pre-stream and their
            # staging precedes the first exps in the ACT queue) ----
            v2aug = v_aug_alloc("v2aug")
            v2raws = [rawt[("v2", j // 2)][:, :, j % 2, :] for j in range(NCH)]
            k_proj_chunk("k1", w_sb["wk1"], bk1_sb, 0)
            k_proj_chunk("k1", w_sb["wk1"], bk1_sb, 1)
            k_proj_chunk("k2", w_sb["wk2"], bk2_sb, 0)
            k_proj_chunk("k2", w_sb["wk2"], bk2_sb, 1)
            k_proj_chunk("k2", w_sb["wk2"], bk2_sb, 2)
            k_proj_chunk("k2", w_sb["wk2"], bk2_sb, 3)
            for _ in range(6):
                nc.tensor.matmul(wps[:, 1, :], warm[:, 0:128], warm[:],
                                 start=True, stop=True)
            # wave-2 raw loads: gate-fill reads each destination region so the
            # transfers start only once the PE reaches this point (~when the
            # initial k1u0/k2u0/v2u0 transfers finish) - they then run at full
            # bandwidth without delaying the first eblk's inputs
            for tg, u, eng in (("k2", 2, nc.gpsimd), ("v2", 1, nc.sync),
                               ("k2", 3, nc.gpsimd), ("v2", 2, nc.sync),
                               ("v2", 3, nc.gpsimd)):
                lw = warm8[:, :] if tg.startswith("v") else warm[0:64, 0:128]
                nc.tensor.matmul(wps[:, 1, 0:256], lw,
                                 rawt[(tg, u)][0:64, 0, 0, 0:256],
                                 start=True, stop=True, tile_position=(0, 0))
                load_unit(tg, u, eng)

            # ---- attention eblk stream ----
            # directions: o2 (kP=k2f, kF=k1f, v2aug), then o1 (swapped)
            v1aug = v_aug_alloc("v1aug")
            dirs = [("o2", "k2", "k1", v2aug, o2Td), ("o1", "k1", "k2", v1aug, o1Td)]
            NE = 2 * NCH * EPC            # 256 eblks
            pss, ebs = {}, {}
            po_cur = [None]

            def eblk_meta(e):
                d = e // (NCH * EPC)
                r = e % (NCH * EPC)
                return d, r // EPC, r % EPC   # direction, chunk j, pair k

            def s_and_exp(e, gate=None, gate_fp8=False):
                d, j, k = eblk_meta(e)
                tag, kPn, kFn, vaug, oTd = dirs[d]
                ps = spsum.tile([128, 2, 512], f32, tag="sAB", name=f"ps_{e}")
                pss[e] = ps
                # HAM filler (start=True score overwrites it; no extra banks;
                # 64-row footprint matches the score pair's row class).
                # Its moving operand optionally reads the destination of a
                # later raw load, WAR-gating that transfer to this eblk.
                # In the steady region the PE itself is the pacer at ~100%
                # duty, so fills only pad the critical path - emit them only
                # in the ramp and where a load gate is needed.
                if gate is not None or e < 24:
                    mv = warm[0:64, 0:256] if gate is None else gate
                    lw = warm8[:, :] if gate_fp8 else warm[0:64, 0:128]
                    nc.tensor.matmul(ps[:, 0, 0:256], lw,
                                     mv, start=True, stop=True,
                                     tile_position=(0, 0))
                for i in range(2):
                    mt = 2 * k + i
                    h = mt % 2
                    nc.tensor.matmul(
                        ps[:, i, :],
                        kf[kPn][h * D:(h + 1) * D, mt * 128:(mt + 1) * 128],
                        kf[kFn][h * D:(h + 1) * D, j * 512:(j + 1) * 512],
                        start=True, stop=True, tile_position=(h * D, 0))
                eb = ep.tile([128, 2, 512], fp8, tag="eblk", name=f"eb_{e}")
                ebs[e] = eb
                if e >= NE - 3:
                    # drain: split halves across both engines to shorten the
                    # critical chain into the last PVs
                    nc.scalar.activation(eb[:, 0, :], ps[:, 0, :], Exp, scale=SCALE)
                    nc.vector.tensor_scalar(eb[:, 1, :].bitcast(i8), ps[:, 1, :],
                                            A_SCH, B_SCH, Alu.mult, Alu.add)
                elif pick_engine(1150.0, 1260.0) == "act":
                    nc.scalar.activation(eb[:], ps[:], Exp, scale=SCALE)
                else:
                    nc.vector.tensor_scalar(eb[:].bitcast(i8), ps[:],
                                            A_SCH, B_SCH, Alu.mult, Alu.add)
                del pss[e]

            def pv(e):
                d, j, k = eblk_meta(e)
                tag, kPn, kFn, vaug, oTd = dirs[d]
                if k == 0:
                    po_cur[0] = opsum.tile([80, 512], f32, tag="po", name=f"po_{d}{j}")
                nc.tensor.matmul(po_cur[0][:], vaug[:, k, :, :], ebs.pop(e)[:],
                                 start=(k == 0), stop=(k == EPC - 1), perf_mode=DR)
                if k == EPC - 1:
                    ot = outp.tile([D + 1, 512], bf16, tag="ot", name=f"ot_{d}{j}")
                    # split halves across both engines: half the displacement
                    # lump on each at the chunk boundary
                    nc.scalar.copy(ot[:, 0:256], po_cur[0][0:D + 1, 0:256])
                    nc.vector.tensor_copy(ot[:, 256:512], po_cur[0][0:D + 1, 256:512])
                    ew["act"] += 450.0
                    ew["dve"] += 470.0
                    oeng = nc.sync if (d == 1 and j >= 6) else nc.gpsimd
                    oeng.dma_start(out=oTd[:, j * 512:(j + 1) * 512], in_=ot[:])

            # hooks: woven raw loads + projections during the stream.
            # e is the eblk index at which the work is EMITTED.
            v1raws = [rawt[("v1", j // 2)][:, :, j % 2, :] for j in range(NCH)]
            def kpc(tg, j):
                names = {"k1": (w_sb["wk1"], bk1_sb), "k2": (w_sb["wk2"], bk2_sb)}
                w, b = names[tg]
                return lambda: k_proj_chunk(tg, w, b, j)

            # WAR-gated raw loads: at eblk e the fill reads the region,
            # then the load is emitted (transfer starts ~at eblk e)
            gated_loads = {
                12: ("k1", 1, nc.sync),
                13: ("v1", 0, nc.gpsimd),
                28: ("k1", 2, nc.sync),
                29: ("v1", 1, nc.gpsimd),
                44: ("k1", 3, nc.sync),
                45: ("v1", 2, nc.gpsimd),
                60: ("v1", 3, nc.gpsimd),
            }
            hooks = {
                1: [lambda: v_proj_unit(v2raws, w_sb["wv2"], v2aug, 0)],
                5: [kpc("k2", 4)],
                6: [lambda: v_proj_unit(v2raws, w_sb["wv2"], v2aug, 1), kpc("k2", 5)],
                9: [kpc("k2", 6)],
                10: [lambda: v_proj_unit(v2raws, w_sb["wv2"], v2aug, 2), kpc("k2", 7)],
                14: [lambda: v_proj_unit(v2raws, w_sb["wv2"], v2aug, 3)],
                26: [kpc("k1", 2)],
                28: [kpc("k1", 3)],
                42: [kpc("k1", 4)],
                44: [kpc("k1", 5)],
                58: [kpc("k1", 6)],
                60: [kpc("k1", 7)],
                103: [lambda: v_proj_unit(v1raws, w_sb["wv1"], v1aug, 0)],
                109: [lambda: v_proj_unit(v1raws, w_sb["wv1"], v1aug, 1)],
                119: [lambda: v_proj_unit(v1raws, w_sb["wv1"], v1aug, 2)],
                124: [lambda: v_proj_unit(v1raws, w_sb["wv1"], v1aug, 3)],
            }

            tfill = [None]
            next_pv = [0]

            def lag_for(e):
                return 10 if e < 48 else LAG

            for e in range(NE + LAG):
                while next_pv[0] <= e - lag_for(e) and next_pv[0] < NE:
                    pv(next_pv[0])
                    next_pv[0] += 1
                if e < NE:
                    for hk in hooks.pop(e, ()):
                        hk()
                    gl = gated_loads.get(e)
                    gate = None
                    gate_fp8 = False
                    if gl is not None:
                        tg, u, _ = gl
                        gate = rawt[(tg, u)][0:64, 0, 0, 0:256]
                        gate_fp8 = tg.startswith("v")
                    s_and_exp(e, gate=gate, gate_fp8=gate_fp8)
                    if gl is not None:
                        load_unit(gl[0], gl[1], gl[2])
                else:
                    # trailing fills keep the PE clock gate open while the
                    # last exps/PVs drain (fresh tile: wps' buffer was
                    # recycled by the score-psum rotation long ago)
                    if tfill[0] is None:
                        tfill[0] = spsum.tile([128, 2, 512], f32, tag="sAB",
                                              name="tail_fill")
                    nc.tensor.matmul(tfill[0][:, 0, :], warm[:, 0:128], warm[:],
                                     start=True, stop=True)
                for hk in hooks.pop(e, ()):
                    hk()

    nc.compile()
    return nc


def _get_nc():
    if "nc" not in _cache:
        _cache["nc"] = _build_module()
    return _cache["nc"]


def kernel(k1, v1, k2, v2,
           wk1_w, wk1_b, wv1_w, wv1_b,
           wk2_w, wk2_b, wv2_w, wv2_b,
           wo1_w, wo1_b, wo2_w, wo2_b):
    import ml_dtypes
    from concourse.bass_utils import run_bass_kernel_spmd

    nc = _get_nc()

    f = np.float32
    bf = ml_dtypes.bfloat16
    f8 = ml_dtypes.float8_e4m3
    k1T = np.ascontiguousarray(np.asarray(k1, f).T).astype(bf)
    v1T = np.ascontiguousarray(np.asarray(v1, f).T).astype(f8)
    k2T = np.ascontiguousarray(np.asarray(k2, f).T).astype(bf)
    v2T = np.ascontiguousarray(np.asarray(v2, f).T).astype(f8)

    in_maps = []
    for h in range(NCORES):
        sl = slice(h * D, (h + 1) * D)

        def dup_w(w):
            ws = np.asarray(w, f)[:, sl]
            return np.ascontiguousarray(np.concatenate([ws, ws], axis=1)).astype(bf)

        def dup_b(b):
            bs = np.asarray(b, f)[sl]
            return np.ascontiguousarray(
                np.concatenate([bs, bs]).reshape(2 * D, 1)).astype(f)

        in_maps.append({
            "k1T": k1T, "v1T": v1T, "k2T": k2T, "v2T": v2T,
            "wk1": dup_w(wk1_w), "wk2": dup_w(wk2_w),
            "wv1": np.ascontiguousarray(np.asarray(wv1_w, f)[:, sl]).astype(f8),
            "wv2": np.ascontiguousarray(np.asarray(wv2_w, f)[:, sl]).astype(f8),
            "bk1": dup_b(wk1_b), "bk2": dup_b(wk2_b),
        })

    res = run_bass_kernel_spmd(nc, in_maps, list(range(NCORES)))
    _cache["last_result"] = res

    o1_all = np.empty((N, 512), f)
    o2_all = np.empty((N, 512), f)
    for h in range(NCORES):
        rh = res.results[h]
        o1t = np.asarray(rh["o1T"], dtype=f)
        o2t = np.asarray(rh["o2T"], dtype=f)
        o1_all[:, h * D:(h + 1) * D] = (o1t[0:D] / o1t[D:D + 1]).T
        o2_all[:, h * D:(h + 1) * D] = (o2t[0:D] / o2t[D:D + 1]).T
    wo1 = np.asarray(wo1_w, f)
    wo2 = np.asarray(wo2_w, f)
    out1 = o1_all @ wo1 + np.asarray(wv1_b, f) @ wo1 + np.asarray(wo1_b, f)
    out2 = o2_all @ wo2 + np.asarray(wv2_b, f) @ wo2 + np.asarray(wo2_b, f)
    return out1, out2


# revision 46
# speedup vs baseline: 1.0174x; 1.0174x over previous
"""MultiHeadDualAttention Trainium2 kernel, v6 (eblk-granular software pipeline).

Sharding: 8 heads -> 8 cores. Each core: full k1/k2 (host-transposed to
[256, 4096] bf16) and v1/v2 (fp8e4 -- the v path tolerates the extra
quantization, relerr 1.57e-2 vs the 2e-2 gate, and it halves that DMA) + its
head's wk slices (column-duplicated on host so the k-projection matmul emits
both 64-row-group copies in one shot) and fp8 wv slices. Outputs per
core: unnormalized o1T/o2T [65, 4096] bf16 (row 64 = softmax denominator);
host divides, applies the wo projection (row-shard of wo = per-head slice,
concat over heads), and adds the v-bias/wo-bias constants (v-bias commutes
through softmax).

Math per head: S[m, n] = k2F[m]·k1F[n] (o2 direction; o1 swaps k1/k2) with
kF the *biased* projections; rowsoftmax / colsoftmax of the shared score
matrix are exactly the reference's two directions.

Perf structure (measured on HW via microbenchmarks):
  - unit of work = eblk: 2 score m-tiles x one 512-wide n-chunk.
    scores: 2 concurrent no-DR fp8 matmuls on 64-row groups (kf stores the
    projection twice, rows 0-63 / 64-127; warm pair issues in ~216ns).
    exp: full-tile [128,2,512] on ACT (exact Exp) or DVE (Schraudolph
    rn(S*a+b) -> int8 bitcast fp8e4), assigned by a greedy load balancer.
    PV: one fp8 DoubleRow full-array matmul (contract 256) accumulating
    [80,512]; row 64 (ones column) = softmax denominator.
  - software pipeline: PV runs 10 eblks behind its scores in the ramp (so a
    v-aug-gated PV cannot block the in-order PE queue), tapering to 5 in
    steady state; score-psum pool of 3 [128,2,512] tiles makes the
    write-after-read distance 3 eblks (the 8 PSUM banks allow no more).
  - steady cadence ~605-615ns/eblk = the ACT/DVE exp floor (both engines
    ~94% busy); the PE absorbs the k1/v1 projections in its slack.
  - HAM (PE clock-gate): warm bursts + ramp-only filler matmuls keep the
    2.4GHz gate engaged; in steady state the real work is dense enough.
    Filler matmuls double as WAR gates that hold back later raw-load
    transfers so early transfers get full HBM bandwidth.
  - wo projection + normalization on host.
"""

import sys

sys.path.insert(0, "/opt/trn_rl_repo")

import numpy as np

N = 4096
C = 256
D = 64
SCALE = float(D) ** -0.5
NCORES = 8
NCH = 8          # n-chunks of 512
EPC = 16         # eblks per chunk (16 m-pairs)
LAG = 5          # PV lag in eblks
A_SCH = float(8.0 * np.log2(np.e) * SCALE)   # schraudolph multiplier
B_SCH = 55.8                                  # schraudolph magic bias

_cache: dict = {}


def _build_module():
    import concourse.bacc as bacc
    import concourse.mybir as mybir
    import concourse.tile as tile

    f32 = mybir.dt.float32
    bf16 = mybir.dt.bfloat16
    fp8 = mybir.dt.float8e4
    i8 = mybir.dt.int8
    Exp = mybir.ActivationFunctionType.Exp
    Ident = mybir.ActivationFunctionType.Identity
    DR = mybir.MatmulPerfMode.DoubleRow
    Alu = mybir.AluOpType

    nc = bacc.Bacc("TRN2", target_bir_lowering=False, debug=False)

    def din(name, shape, dt=bf16):
        return nc.dram_tensor(name, shape, dt, kind="ExternalInput").ap()

    def dout(name, shape, dt):
        return nc.dram_tensor(name, shape, dt, kind="ExternalOutput").ap()

    k1T = din("k1T", [C, N])
    v1T = din("v1T", [C, N], mybir.dt.float8e4)
    k2T = din("k2T", [C, N])
    v2T = din("v2T", [C, N], mybir.dt.float8e4)
    wk1 = din("wk1", [C, 2 * D])          # column-duplicated on host
    wk2 = din("wk2", [C, 2 * D])
    wv1 = din("wv1", [C, D], mybir.dt.float8e4)
    wv2 = din("wv2", [C, D], mybir.dt.float8e4)
    bk1 = din("bk1", [2 * D, 1], f32)     # row-duplicated on host
    bk2 = din("bk2", [2 * D, 1], f32)

    o1Td = dout("o1T", [D + 1, N], bf16)
    o2Td = dout("o2T", [D + 1, N], bf16)

    # elementwise-engine load balancer (ns estimates from microbench)
    ew = {"act": 0.0, "dve": 0.0}

    def pick_engine(act_cost, dve_cost):
        if ew["act"] + act_cost <= ew["dve"] + dve_cost:
            ew["act"] += act_cost
            return "act"
        ew["dve"] += dve_cost
        return "dve"

    with tile.TileContext(nc) as tc:
        with (
            tc.tile_pool(name="const", bufs=1) as constp,
            tc.tile_pool(name="eblk", bufs=14) as ep,
            tc.tile_pool(name="outp", bufs=3) as outp,
            tc.tile_pool(name="spsum", bufs=3, space="PSUM") as spsum,
            tc.tile_pool(name="opsum", bufs=2, space="PSUM") as opsum,
        ):
            # ---- weights (gpsimd hwdge queue; SP queue is for raw loads) ----
            w_sb = {}
            _pending_w = []
            for name, drt, cols in (("wk1", wk1, 2 * D), ("wk2", wk2, 2 * D),
                                    ("wv1", wv1, D), ("wv2", wv2, D)):
                wdt = fp8 if name.startswith("wv") else bf16
                t = constp.tile([128, 2, cols], wdt, tag=name, name=f"w_{name}")
                for ct in range(2):
                    _pending_w.append((t[:, ct, :], drt[ct * 128:(ct + 1) * 128, :]))
                w_sb[name] = t
            bk1_sb = constp.tile([2 * D, 1], f32, tag="bk1")
            bk2_sb = constp.tile([2 * D, 1], f32, tag="bk2")

            warm = constp.tile([128, 512], bf16, tag="warm")
            nc.vector.memset(warm[:], 0.0)
            warm8 = constp.tile([64, 128], fp8, tag="warm8")
            nc.vector.memset(warm8[:], 0.0)

            # ---- raw tensors: one tile PER 2-chunk UNIT (tile-granular
            # dependency tracking would otherwise serialize gated loads of
            # the same tensor); c = ct*128 + p ----
            rawt = {(tg, u): constp.tile([128, 2, 2, 512],
                                         fp8 if tg.startswith("v") else bf16,
                                         tag=f"{tg}raw{u}", name=f"rawt_{tg}{u}")
                    for tg in ("k1", "k2", "v2", "v1") for u in range(NCH // 2)}
            rawd = {"k1": k1T, "k2": k2T, "v2": v2T, "v1": v1T}

            def load_unit(tg, u, eng):
                eng.dma_start(
                    out=rawt[(tg, u)][:],
                    in_=rawd[tg][:, (2 * u) * 512:(2 * u + 2) * 512].rearrange(
                        "(c p) (j n) -> p c j n", c=2, j=2))

            # raw loads split across TWO hwdge queues (SP + gpsimd): one
            # queue only sustains ~half the per-core HBM bandwidth, and
            # transfers within a queue share bandwidth round-robin — so only
            # the first-consumed units go in the initial batch; k1u1-3/v1
            # are emitted later on the gpsimd queue BEHIND the out-DMAs,
            # which gates their transfer start into the attention stream.
            for dst, srcw in _pending_w:
                nc.gpsimd.dma_start(out=dst, in_=srcw)
            nc.gpsimd.dma_start(out=bk1_sb[:], in_=bk1[:])
            nc.gpsimd.dma_start(out=bk2_sb[:], in_=bk2[:])
            load_unit("k1", 0, nc.sync)
            load_unit("k2", 0, nc.gpsimd)
            load_unit("v2", 0, nc.gpsimd)
            load_unit("k2", 1, nc.sync)
            # all other raw units are emitted inside the attention stream,
            # WAR-gated by a fill matmul that reads their destination region
            # (see s_and_exp gate), so early transfers get the full bandwidth

            # ---- kf staging: [128, N] fp8, rows 0-63 / 64-127 identical ----
            kf = {"k1": constp.tile([128, N], fp8, tag="k1f", name="kf_k1"),
                  "k2": constp.tile([128, N], fp8, tag="k2f", name="kf_k2")}

            def k_proj_chunk(tg, w, b_sb, j):
                """Project chunk j of tg into kf[tg] (both row copies). Uses an
                opsum-pool tile so the score-psum pool is never borrowed."""
                ps = opsum.tile([128, 512], f32, tag="po", name=f"kp_{tg}{j}")
                for ct in range(2):
                    nc.tensor.matmul(
                        ps[:], w[:, ct, :], rawt[(tg, j // 2)][:, ct, j % 2, :],
                        start=(ct == 0), stop=(ct == 1))
                nc.scalar.activation(
                    kf[tg][:, j * 512:(j + 1) * 512], ps[:], Ident, bias=b_sb[:])
                ew["act"] += 700.0

            # ---- v projections -> fp8 v-aug [128, pair, kt, 80] ----
            # per-unit granularity (2 chunks -> pairs 4u..4u+3) so v-aug
            # tracks the raw-load pipeline
            def v_proj_unit(raws, w, vaug, u):
                vps = opsum.tile([128, 512], f32, tag="po",
                                 name=f"vp_{id(vaug)}_{u}")
                for jj in range(2):
                    raw = raws[2 * u + jj]
                    for k in range(4):
                        nt_loc = jj * 4 + k
                        out = vps[:, nt_loc * D:(nt_loc + 1) * D]
                        for ct in range(2):
                            nc.tensor.matmul(
                                out, raw[:, ct, k * 128:(k + 1) * 128], w[:, ct, :],
                                start=(ct == 0), stop=(ct == 1))
                nc.vector.tensor_copy(
                    vaug[:, 4 * u:4 * (u + 1), :, 0:D], vps[:])
                ew["dve"] += 730.0

            def v_aug_alloc(tag):
                vaug = constp.tile([128, 16, 2, 80], fp8, tag=tag, name=f"vaug_{tag}")
                nc.vector.memset(vaug[:, :, :, D:80], 0.0)
                nc.vector.memset(vaug[:, :, :, D:D + 1], 1.0)
                return vaug

            # ---- HAM warm burst: dependency-free, issues during the raw
            # loads so the clock gate is open when eblk 0 starts ----
            wps = spsum.tile([128, 2, 512], f32, tag="sAB", name="warm_att")
            for _ in range(15):
                nc.tensor.matmul(wps[:, 0, :], warm[:, 0:128], warm[:],
                                 start=True, stop=True)

            # ---- pre-phase: eblk 0-7 inputs (k2u1 lands ~13us, before the
            # PE clears the warm bursts, so c2/c3 project pre-stream and their
            # staging precedes the first exps in the ACT queue) ----
            v2aug = v_aug_alloc("v2aug")
            v2raws = [rawt[("v2", j // 2)][:, :, j % 2, :] for j in range(NCH)]
            k_proj_chunk("k1", w_sb["wk1"], bk1_sb, 0)
            k_proj_chunk("k1", w_sb["wk1"], bk1_sb, 1)
            k_proj_chunk("k2", w_sb["wk2"], bk2_sb, 0)
            k_proj_chunk("k2", w_sb["wk2"], bk2_sb, 1)
            for _ in range(6):
                nc.tensor.matmul(wps[:, 1, :], warm[:, 0:128], warm[:],
                                 start=True, stop=True)
            # wave-2 raw loads: gate-fill reads each destination region so the
            # transfers start only once the PE reaches this point (~when the
            # initial k1u0/k2u0/v2u0 transfers finish) - they then run at full
            # bandwidth without delaying the first eblk's inputs
            for tg, u, eng in (("k2", 2, nc.gpsimd), ("v2", 1, nc.sync),
                               ("k2", 3, nc.gpsimd), ("v2", 2, nc.sync),
                               ("v2", 3, nc.gpsimd)):
                if eng is nc.sync:
                    # sync-queue wave shares the queue with k2u1 (needed by
                    # eblk 4): RAW-anchor on its region so these transfers
                    # cannot start until k2u1 has fully landed. The v-region
                    # read gives the usual WAR gate, done fp8-vs-fp8 via a
                    # separate read MM (dtype must match lhsT).
                    nc.tensor.matmul(wps[:, 1, 0:256], warm[0:64, 0:128],
                                     rawt[("k2", 1)][0:64, 0, 0, 0:256],
                                     start=True, stop=True, tile_position=(0, 0))
                    nc.tensor.matmul(wps[:, 1, 0:256], warm8[:, :],
                                     rawt[(tg, u)][0:64, 0, 0, 0:256],
                                     start=True, stop=True, tile_position=(0, 0))
                else:
                    lw = warm8[:, :] if tg.startswith("v") else warm[0:64, 0:128]
                    nc.tensor.matmul(wps[:, 1, 0:256], lw,
                                     rawt[(tg, u)][0:64, 0, 0, 0:256],
                                     start=True, stop=True, tile_position=(0, 0))
                load_unit(tg, u, eng)

            # ---- attention eblk stream ----
            # directions: o2 (kP=k2f, kF=k1f, v2aug), then o1 (swapped)
            v1aug = v_aug_alloc("v1aug")
            dirs = [("o2", "k2", "k1", v2aug, o2Td), ("o1", "k1", "k2", v1aug, o1Td)]
            NE = 2 * NCH * EPC            # 256 eblks
            pss, ebs = {}, {}
            po_cur = [None]

            def eblk_meta(e):
                d = e // (NCH * EPC)
                r = e % (NCH * EPC)
                return d, r // EPC, r % EPC   # direction, chunk j, pair k

            def s_and_exp(e, gate=None, gate_fp8=False):
                d, j, k = eblk_meta(e)
                tag, kPn, kFn, vaug, oTd = dirs[d]
                ps = spsum.tile([128, 2, 512], f32, tag="sAB", name=f"ps_{e}")
                pss[e] = ps
                # HAM filler (start=True score overwrites it; no extra banks;
                # 64-row footprint matches the score pair's row class).
                # Its moving operand optionally reads the destination of a
                # later raw load, WAR-gating that transfer to this eblk.
                # In the steady region the PE itself is the pacer at ~100%
                # duty, so fills only pad the critical path - emit them only
                # in the ramp and where a load gate is needed.
                if gate is not None or e < 24:
                    mv = warm[0:64, 0:256] if gate is None else gate
                    lw = warm8[:, :] if gate_fp8 else warm[0:64, 0:128]
                    nc.tensor.matmul(ps[:, 0, 0:256], lw,
                                     mv, start=True, stop=True,
                                     tile_position=(0, 0))
                for i in range(2):
                    mt = 2 * k + i
                    h = mt % 2
                    nc.tensor.matmul(
                        ps[:, i, :],
                        kf[kPn][h * D:(h + 1) * D, mt * 128:(mt + 1) * 128],
                        kf[kFn][h * D:(h + 1) * D, j * 512:(j + 1) * 512],
                        start=True, stop=True, tile_position=(h * D, 0))
                eb = ep.tile([128, 2, 512], fp8, tag="eblk", name=f"eb_{e}")
                ebs[e] = eb
                if e >= NE - 3:
                    # drain: split halves across both engines to shorten the
                    # critical chain into the last PVs
                    nc.scalar.activation(eb[:, 0, :], ps[:, 0, :], Exp, scale=SCALE)
                    nc.vector.tensor_scalar(eb[:, 1, :].bitcast(i8), ps[:, 1, :],
                                            A_SCH, B_SCH, Alu.mult, Alu.add)
                elif pick_engine(1150.0, 1260.0) == "act":
                    nc.scalar.activation(eb[:], ps[:], Exp, scale=SCALE)
                else:
                    nc.vector.tensor_scalar(eb[:].bitcast(i8), ps[:],
                                            A_SCH, B_SCH, Alu.mult, Alu.add)
                del pss[e]

            def pv(e):
                d, j, k = eblk_meta(e)
                tag, kPn, kFn, vaug, oTd = dirs[d]
                if k == 0:
                    po_cur[0] = opsum.tile([80, 512], f32, tag="po", name=f"po_{d}{j}")
                nc.tensor.matmul(po_cur[0][:], vaug[:, k, :, :], ebs.pop(e)[:],
                                 start=(k == 0), stop=(k == EPC - 1), perf_mode=DR)
                if k == EPC - 1:
                    ot = outp.tile([D + 1, 512], bf16, tag="ot", name=f"ot_{d}{j}")
                    # split halves across both engines: half the displacement
                    # lump on each at the chunk boundary
                    nc.scalar.copy(ot[:, 0:256], po_cur[0][0:D + 1, 0:256])
                    nc.vector.tensor_copy(ot[:, 256:512], po_cur[0][0:D + 1, 256:512])
                    ew["act"] += 450.0
                    ew["dve"] += 470.0
                    oeng = nc.sync if (d == 1 and j >= 6) else nc.gpsimd
                    oeng.dma_start(out=oTd[:, j * 512:(j + 1) * 512], in_=ot[:])

            # hooks: woven raw loads + projections during the stream.
            # e is the eblk index at which the work is EMITTED.
            v1raws = [rawt[("v1", j // 2)][:, :, j % 2, :] for j in range(NCH)]
            def kpc(tg, j):
                names = {"k1": (w_sb["wk1"], bk1_sb), "k2": (w_sb["wk2"], bk2_sb)}
                w, b = names[tg]
                return lambda: k_proj_chunk(tg, w, b, j)

            # WAR-gated raw loads: at eblk e the fill reads the region,
            # then the load is emitted (transfer starts ~at eblk e)
            gated_loads = {
                12: ("k1", 1, nc.sync),
                13: ("v1", 0, nc.gpsimd),
                28: ("k1", 2, nc.sync),
                29: ("v1", 1, nc.gpsimd),
                44: ("k1", 3, nc.sync),
                45: ("v1", 2, nc.gpsimd),
                60: ("v1", 3, nc.gpsimd),
            }
            hooks = {
                1: [lambda: v_proj_unit(v2raws, w_sb["wv2"], v2aug, 0)],
                2: [kpc("k2", 2)],
                3: [kpc("k2", 3)],
                5: [kpc("k2", 4)],
                6: [lambda: v_proj_unit(v2raws, w_sb["wv2"], v2aug, 1), kpc("k2", 5)],
                9: [kpc("k2", 6)],
                10: [lambda: v_proj_unit(v2raws, w_sb["wv2"], v2aug, 2), kpc("k2", 7)],
                14: [lambda: v_proj_unit(v2raws, w_sb["wv2"], v2aug, 3)],
                26: [kpc("k1", 2)],
                28: [kpc("k1", 3)],
                42: [kpc("k1", 4)],
                44: [kpc("k1", 5)],
                58: [kpc("k1", 6)],
                60: [kpc("k1", 7)],
                103: [lambda: v_proj_unit(v1raws, w_sb["wv1"], v1aug, 0)],
                109: [lambda: v_proj_unit(v1raws, w_sb["wv1"], v1aug, 1)],
                119: [lambda: v_proj_unit(v1raws, w_sb["wv1"], v1aug, 2)],
                124: [lambda: v_proj_unit(v1raws, w_sb["wv1"], v1aug, 3)],
            }

            tfill = [None]
            next_pv = [0]

            def lag_for(e):
                return 10 if e < 48 else LAG

            for e in range(NE + LAG):
                while next_pv[0] <= e - lag_for(e) and next_pv[0] < NE:
                    pv(next_pv[0])
                    next_pv[0] += 1
                if e < NE:
                    for hk in hooks.pop(e, ()):
                        hk()
                    gl = gated_loads.get(e)
                    gate = None
                    gate_fp8 = False
                    if gl is not None:
                        tg, u, _ = gl
                        gate = rawt[(tg, u)][0:64, 0, 0, 0:256]
                        gate_fp8 = tg.startswith("v")
                    s_and_exp(e, gate=gate, gate_fp8=gate_fp8)
                    if gl is not None:
                        load_unit(gl[0], gl[1], gl[2])
                else:
                    # trailing fills keep the PE clock gate open while the
                    # last exps/PVs drain (fresh tile: wps' buffer was
                    # recycled by the score-psum rotation long ago)
                    if tfill[0] is None:
                        tfill[0] = spsum.tile([128, 2, 512], f32, tag="sAB",
                                              name="tail_fill")
                    nc.tensor.matmul(tfill[0][:, 0, :], warm[:, 0:128], warm[:],
                                     start=True, stop=True)
                for hk in hooks.pop(e, ()):
                    hk()

    nc.compile()
    return nc


def _get_nc():
    if "nc" not in _cache:
        _cache["nc"] = _build_module()
    return _cache["nc"]


def kernel(k1, v1, k2, v2,
           wk1_w, wk1_b, wv1_w, wv1_b,
           wk2_w, wk2_b, wv2_w, wv2_b,
           wo1_w, wo1_b, wo2_w, wo2_b):
    import ml_dtypes
    from concourse.bass_utils import run_bass_kernel_spmd

    nc = _get_nc()

    f = np.float32
    bf = ml_dtypes.bfloat16
    f8 = ml_dtypes.float8_e4m3
    k1T = np.ascontiguousarray(np.asarray(k1, f).T).astype(bf)
    v1T = np.ascontiguousarray(np.asarray(v1, f).T).astype(f8)
    k2T = np.ascontiguousarray(np.asarray(k2, f).T).astype(bf)
    v2T = np.ascontiguousarray(np.asarray(v2, f).T).astype(f8)

    in_maps = []
    for h in range(NCORES):
        sl = slice(h * D, (h + 1) * D)

        def dup_w(w):
            ws = np.asarray(w, f)[:, sl]
            return np.ascontiguousarray(np.concatenate([ws, ws], axis=1)).astype(bf)

        def dup_b(b):
            bs = np.asarray(b, f)[sl]
            return np.ascontiguousarray(
                np.concatenate([bs, bs]).reshape(2 * D, 1)).astype(f)

        in_maps.append({
            "k1T": k1T, "v1T": v1T, "k2T": k2T, "v2T": v2T,
            "wk1": dup_w(wk1_w), "wk2": dup_w(wk2_w),
            "wv1": np.ascontiguousarray(np.asarray(wv1_w, f)[:, sl]).astype(f8),
            "wv2": np.ascontiguousarray(np.asarray(wv2_w, f)[:, sl]).astype(f8),
            "bk1": dup_b(wk1_b), "bk2": dup_b(wk2_b),
        })

    res = run_bass_kernel_spmd(nc, in_maps, list(range(NCORES)))
    _cache["last_result"] = res

    o1_all = np.empty((N, 512), f)
    o2_all = np.empty((N, 512), f)
    for h in range(NCORES):
        rh = res.results[h]
        o1t = np.asarray(rh["o1T"], dtype=f)
        o2t = np.asarray(rh["o2T"], dtype=f)
        o1_all[:, h * D:(h + 1) * D] = (o1t[0:D] / o1t[D:D + 1]).T
        o2_all[:, h * D:(h + 1) * D] = (o2t[0:D] / o2t[D:D + 1]).T
    wo1 = np.asarray(wo1_w, f)
    wo2 = np.asarray(wo2_w, f)
    out1 = o1_all @ wo1 + np.asarray(wv1_b, f) @ wo1 + np.asarray(wo1_b, f)
    out2 = o2_all @ wo2 + np.asarray(wv2_b, f) @ wo2 + np.asarray(wo2_b, f)
    return out1, out2
